# revision 47
# baseline (speedup 1.0000x reference)
"""Trainium2 Bass kernel for a CrossAttentionBlock (GroupNorm + 8-head
cross-attention + output projection + residual).

Sharding: one attention head per NeuronCore (8 heads / 8 cores).  Each core
computes its head's partial output projection wo[:, h] @ attn_h; the host sums
the 8 partials (partial-sum unshard).  Residual and output bias are added on
core 0 only (mask input), so the host-side sum is a pure reduce.

Structure (single TileContext, dependency-scheduled):
 - GroupNorm is folded into a per-channel scale/offset applied to the Q
   projection weights (q = (wq*a) @ x + (wq@d + bq)).
 - All heavy matmuls run in float32r (full PE rate).  x/ctx are declared
   float32r in DRAM so the DMA load is the rounded producer (no cast passes).
 - Scores are computed transposed (T[j,i] = k_j . q_i, row-packed K=64
   pairs), exp on ScalarE, and the softmax denominator rides as a ones-column
   in the augmented V^T of the AV matmul.
 - K/V are produced in four column-quarters and the attention chunk work is
   emitted diagonally against the quarters so the ScalarE exp pipeline starts
   ~40us in and stays saturated.

Self-contained: hardcodes all shapes from the problem spec.
"""

import sys

sys.path.insert(0, "/opt/trn_rl_repo")

import numpy as np

import concourse.bass as bass
import concourse.tile as tile
from concourse import bacc, mybir

F32 = mybir.dt.float32
F32R = mybir.dt.float32r

CH = 512          # x channels
CTXC = 768        # context channels
N = 4096          # spatial positions (64*64)
NH = 8            # heads
DH = 64           # head dim
G = 32            # groupnorm groups
EPS = 1e-5
NCO = CH // 128   # x channel blocks (4)
NCK = CTXC // 128  # ctx channel blocks (6)
IC = 512          # query-chunk size
NIC = N // IC     # 8 query chunks
NJT = N // 128    # 32 key tiles
NQT = 4           # context column quarters
QW = N // NQT     # quarter width (1024)
JPQ = NJT // 2 // NQT  # key-tile pairs per quarter (4)
SCALE = 1.0 / 8.0  # 1/sqrt(DH)
# PROJ_F32: run Q/KV projections as plain fp32 matmuls (4 cyc/row on the PE,
# but exact) instead of f32r with raw DMA-loaded operands (full rate, but the
# PE rounds raw operands to 11 mantissa bits).
PROJ_F32 = False
# PROJ_CAST: cast x/ctx blocks to f32r with compute engines (Pool for x, DVE
# for ctx) before the f32r projection matmuls — engine-produced f32r operands
# take the full-precision PE path (raw DMA-loaded operands get rounded to 11
# mantissa bits).
PROJ_CAST = True

ADD = mybir.AluOpType.add
SUB = mybir.AluOpType.subtract
MUL = mybir.AluOpType.mult


def build_nc():
    nc = bacc.Bacc("TRN2", num_devices=8, debug=False)

    XDT = F32 if (PROJ_F32 or PROJ_CAST) else F32R
    x = nc.dram_tensor("x", (CH, N), XDT, kind="ExternalInput")
    ctx_t = nc.dram_tensor("ctx", (CTXC, N), XDT, kind="ExternalInput")
    gn_w = nc.dram_tensor("gn_w", (CH,), F32, kind="ExternalInput")
    gn_b = nc.dram_tensor("gn_b", (CH,), F32, kind="ExternalInput")
    wqT = nc.dram_tensor("wqT", (CH, DH), F32, kind="ExternalInput")
    wkvT = nc.dram_tensor("wkvT", (CTXC, 2 * DH), F32, kind="ExternalInput")
    woT = nc.dram_tensor("woT", (DH, CH), F32, kind="ExternalInput")
    bq = nc.dram_tensor("bq", (DH,), F32, kind="ExternalInput")
    bk = nc.dram_tensor("bk", (DH,), F32, kind="ExternalInput")
    bv = nc.dram_tensor("bv", (DH,), F32, kind="ExternalInput")
    bo = nc.dram_tensor("bo", (CH,), F32, kind="ExternalInput")
    is0 = nc.dram_tensor("is0", (1, 1), F32, kind="ExternalInput")
    gmat = nc.dram_tensor("gmat", (128, 8), F32, kind="ExternalInput")
    gmatT = nc.dram_tensor("gmatT", (8, 128), F32, kind="ExternalInput")
    ident_d = nc.dram_tensor("ident", (128, 128), F32, kind="ExternalInput")
    partial = nc.dram_tensor("partial", (CH, N), F32, kind="ExternalOutput")

    xv = x.rearrange("(co p) n -> p co n", p=128)
    pvw = partial.rearrange("(co p) n -> p co n", p=128)

    with tile.TileContext(nc) as tc:
        singles = tc.alloc_tile_pool(name="singles", bufs=1)
        dram = tc.alloc_tile_pool(name="dram", bufs=1, space="DRAM")
        # PSUM: pst 3 + pav 2 + pso 1 + bsh 2 = 8 banks
        pst = tc.alloc_tile_pool(name="pst", bufs=2, space="PSUM")
        psav = tc.alloc_tile_pool(name="psav", bufs=2, space="PSUM")
        bsh = tc.alloc_tile_pool(name="bsh", bufs=2, space="PSUM")
        expp = tc.alloc_tile_pool(name="expp", bufs=4)
        attp = tc.alloc_tile_pool(name="attp", bufs=2)
        outp = tc.alloc_tile_pool(name="outp", bufs=2)
        xmp = tc.alloc_tile_pool(name="xmp", bufs=2)
        ctxq = tc.alloc_tile_pool(name="ctxq", bufs=6)
        stats_pool = tc.alloc_tile_pool(name="stats", bufs=2)
        xrp = tc.alloc_tile_pool(name="xrp", bufs=2)
        crp = tc.alloc_tile_pool(name="crp", bufs=2)

        # ---------------- persistent SBUF tiles ----------------
        x_sb = singles.tile([128, NCO, N], XDT)
        q_dup = singles.tile([128, N], F32R)
        k_dup = singles.tile([128, N], F32R)
        v_sb = singles.tile([128, N], F32)       # only rows 64:128 used
        vT_aug = singles.tile([128, NJT, DH + 1], F32R)
        wqT_sb = singles.tile([128, NCO, DH], F32)
        wqs_sb = singles.tile([128, NCO, DH], F32 if PROJ_F32 else F32R)
        assert not (PROJ_F32 and PROJ_CAST)
        wkvT_sb = singles.tile([128, NCK, 2 * DH], F32)
        wkv_r = singles.tile([128, NCK, 2 * DH], F32 if PROJ_F32 else F32R)
        woT_sb = singles.tile([DH, CH], F32)
        woT_r = singles.tile([DH, CH], F32R)
        ident = singles.tile([128, 128], F32)
        ones1 = singles.tile([1, DH], F32R)
        gmat_sb = singles.tile([128, 8], F32)
        gmatT_sb = singles.tile([8, 128], F32)
        gnw_pc = singles.tile([128, NCO], F32)
        gnb_pc = singles.tile([128, NCO], F32)
        mvall = singles.tile([128, NCO, 2], F32)
        mv3 = singles.tile([128, NCO, 3], F32)
        gsm = singles.tile([8, NCO, 3], F32)
        gmu84 = singles.tile([8, NCO], F32)
        gvar84 = singles.tile([8, NCO], F32)
        srt84 = singles.tile([8, NCO], F32)
        grs = singles.tile([8, NCO, 2], F32)
        rg_pc = singles.tile([128, NCO, 2], F32)
        tmp_pc = singles.tile([128, NCO], F32)
        eps8 = singles.tile([8, 1], F32)
        a_pc = singles.tile([128, NCO], F32)
        d_pc = singles.tile([128, NCO], F32)
        qbias = singles.tile([DH, 1], F32)
        bq_sb = singles.tile([DH, 1], F32)
        kb_sb = singles.tile([DH, 1], F32)
        bv64 = singles.tile([DH, 1], F32)
        wobv_pc = singles.tile([128, NCO], F32)
        bomv = singles.tile([128, NCO], F32)
        bo_pc = singles.tile([128, NCO], F32)
        bom = singles.tile([128, NCO], F32)
        msk = singles.tile([128, 1], F32)

        # ---------------- input loads ----------------
        for co in range(NCO):
            nc.sync.dma_start(out=x_sb[:, co, :], in_=xv[:, co, :])
        nc.sync.dma_start(out=wqT_sb[:], in_=wqT.rearrange("(co p) d -> p co d", p=128))
        nc.sync.dma_start(out=wkvT_sb[:], in_=wkvT.rearrange("(ck p) d -> p ck d", p=128))
        nc.sync.dma_start(out=woT_sb[:], in_=woT[:])
        nc.sync.dma_start(out=gnw_pc[:], in_=gn_w.rearrange("(co p) -> p co", p=128))
        nc.sync.dma_start(out=gnb_pc[:], in_=gn_b.rearrange("(co p) -> p co", p=128))
        nc.sync.dma_start(out=gmat_sb[:], in_=gmat[:])
        nc.sync.dma_start(out=gmatT_sb[:], in_=gmatT[:])
        nc.sync.dma_start(out=ident[:], in_=ident_d[:])
        nc.sync.dma_start(out=bq_sb[:], in_=bq[:][:, None])
        nc.sync.dma_start(out=kb_sb[:], in_=bk[:][:, None])
        nc.sync.dma_start(out=bv64[:], in_=bv[:][:, None])
        nc.sync.dma_start(out=bo_pc[:], in_=bo.rearrange("(co p) -> p co", p=128))
        nc.sync.dma_start(out=msk[:], in_=is0[:].to_broadcast((128, 1)))

        onesf = singles.tile([128, 64], F32)
        nc.vector.memset(onesf[:], 1.0)
        expwarm = singles.tile([1, 1], F32)
        nc.vector.memset(expwarm[:], 1.0)
        nc.scalar.activation(out=expwarm[:], in_=expwarm[:],
                             func=mybir.ActivationFunctionType.Ln)
        nc.scalar.activation(out=expwarm[:], in_=expwarm[:],
                             func=mybir.ActivationFunctionType.Exp)
        nc.vector.tensor_copy(out=ones1[:], in_=onesf[0:1, 0:DH])
        nc.vector.tensor_copy(out=vT_aug[:, :, DH], in_=onesf[:, 0:NJT])
        nc.vector.memset(eps8[:], EPS)
        nc.vector.tensor_scalar_mul(out=bom[:], in0=bo_pc[:], scalar1=msk[:])
        nc.vector.tensor_copy(out=woT_r[:], in_=woT_sb[:])
        nc.vector.tensor_copy(out=wkv_r[:], in_=wkvT_sb[:])
        for ot in range(NCO):
            pwb = bsh.tile([128, 1], F32, tag="b", name="pwb")
            nc.tensor.matmul(pwb[:], woT_sb[:, ot * 128:(ot + 1) * 128], bv64[:],
                             start=True, stop=True)
            nc.vector.tensor_copy(out=wobv_pc[:, ot:ot + 1], in_=pwb[:])
        nc.vector.tensor_tensor(out=bomv[:], in0=bom[:], in1=wobv_pc[:], op=ADD)

        # ---------------- groupnorm statistics ----------------
        for co in range(NCO):
            st = stats_pool.tile([128, 8, 6], F32)
            xvw = x_sb[:, co, :].bitcast(F32).rearrange("p (s c) -> p s c", c=512)
            for s in range(8):
                nc.vector.bn_stats(out=st[:, s, :], in_=xvw[:, s, :])
            nc.vector.bn_aggr(out=mvall[:, co, :], in_=st[:])
        # group stats via tiny PE matmuls (no DRAM bounces):
        # gsm[j, co, :] = sum over the 16 channels of group (co*8+j) of
        # (mean, var, mean^2); then per-group mu/var and per-channel a/d.
        for co in range(NCO):
            nc.vector.tensor_copy(out=mv3[:, co, 0:2], in_=mvall[:, co, :])
            nc.vector.tensor_tensor(out=mv3[:, co, 2:3], in0=mvall[:, co, 0:1],
                                    in1=mvall[:, co, 0:1], op=MUL)
            psg = bsh.tile([8, 3], F32, tag="b", name="psg")
            nc.tensor.matmul(psg[:], gmat_sb[:], mv3[:, co, :],
                             start=True, stop=True)
            nc.vector.tensor_copy(out=gsm[:, co, :], in_=psg[:])
        nc.vector.tensor_scalar_mul(out=gmu84[:], in0=gsm[:, :, 0], scalar1=1.0 / 16.0)
        # gvar = E[var] + E[mean^2] - gmu^2
        nc.vector.tensor_tensor(out=gvar84[:], in0=gsm[:, :, 1], in1=gsm[:, :, 2],
                                op=ADD)
        nc.vector.tensor_scalar_mul(out=gvar84[:], in0=gvar84[:], scalar1=1.0 / 16.0)
        nc.vector.tensor_tensor(out=srt84[:], in0=gmu84[:], in1=gmu84[:], op=MUL)
        nc.vector.tensor_tensor(out=gvar84[:], in0=gvar84[:], in1=srt84[:], op=SUB)
        # rstd = exp(-0.5*ln(var+eps)): keeps the ScalarE activation table
        # on a set that also holds Exp.
        nc.scalar.activation(out=srt84[:], in_=gvar84[:],
                             func=mybir.ActivationFunctionType.Ln, bias=eps8[:])
        nc.scalar.activation(out=grs[:, :, 0], in_=srt84[:],
                             func=mybir.ActivationFunctionType.Exp, scale=-0.5)
        nc.vector.tensor_copy(out=grs[:, :, 1], in_=gmu84[:])
        for co in range(NCO):
            psr = bsh.tile([128, 2], F32, tag="b", name="psr")
            nc.tensor.matmul(psr[:], gmatT_sb[:], grs[:, co, :],
                             start=True, stop=True)
            nc.vector.tensor_copy(out=rg_pc[:, co, :], in_=psr[:])
        nc.vector.tensor_tensor(out=a_pc[:], in0=gnw_pc[:], in1=rg_pc[:, :, 0],
                                op=MUL)
        nc.vector.tensor_tensor(out=tmp_pc[:], in0=rg_pc[:, :, 1], in1=a_pc[:], op=MUL)
        nc.vector.tensor_tensor(out=d_pc[:], in0=gnb_pc[:], in1=tmp_pc[:], op=SUB)

        # qbias = wq_h @ d + bq ; wqs = wqT * a (columns scaled per channel)
        qb = psav.tile([DH, 1], F32, tag="pav", name="qb")
        for co in range(NCO):
            nc.tensor.matmul(qb[:], wqT_sb[:, co, :], d_pc[:, co:co + 1],
                             start=(co == 0), stop=(co == NCO - 1))
        nc.vector.tensor_tensor(out=qbias[:], in0=qb[:], in1=bq_sb[:], op=ADD)
        for co in range(NCO):
            nc.vector.tensor_scalar_mul(out=wqs_sb[:, co, :], in0=wqT_sb[:, co, :],
                                        scalar1=a_pc[:, co:co + 1])

        # ---------------- projections (f32r) ----------------
        def q_proj(ic, pool=None, tag="b"):
            sl = slice(ic * IC, (ic + 1) * IC)
            pq = (pool or bsh).tile([DH, IC], F32, tag=tag, name="pq")
            for co in range(NCO):
                if PROJ_CAST:
                    xr = xrp.tile([128, IC], F32R, tag="xr", name="xr")
                    nc.gpsimd.tensor_copy(out=xr[:], in_=x_sb[:, co, sl])
                    rhs = xr[:]
                else:
                    rhs = x_sb[:, co, sl]
                nc.tensor.matmul(pq[:], wqs_sb[:, co, :], rhs,
                                 start=(co == 0), stop=(co == NCO - 1))
            # q = (wq@xn + qbias) * SCALE, duplicated to partitions 64:128
            nc.vector.tensor_scalar(out=q_dup[0:64, sl], in0=pq[:],
                                    scalar1=qbias[:], scalar2=SCALE,
                                    op0=ADD, op1=MUL)
            nc.gpsimd.tensor_copy(out=q_dup[64:128, sl],
                                  in_=q_dup[0:64, sl].bitcast(F32))

        def kv_quarter(qt):
            cts = []
            for ck in range(NCK):
                ct = ctxq.tile([128, QW], XDT, tag="ctq", name="ctq")
                nc.sync.dma_start(
                    out=ct[:],
                    in_=ctx_t[ck * 128:(ck + 1) * 128, qt * QW:(qt + 1) * QW])
                cts.append(ct)
            kvs = []
            for lc in range(QW // IC):
                ic = qt * (QW // IC) + lc
                sl = slice(ic * IC, (ic + 1) * IC)
                kv = bsh.tile([128, IC], F32, tag="b", name="kv")
                kvs.append((kv, sl))
                for ck in range(NCK):
                    if PROJ_CAST:
                        cr = crp.tile([128, IC], F32R, tag="cr", name="cr")
                        nc.vector.tensor_copy(out=cr[:],
                                              in_=cts[ck][:, lc * IC:(lc + 1) * IC])
                        rhs = cr[:]
                    else:
                        rhs = cts[ck][:, lc * IC:(lc + 1) * IC]
                    nc.tensor.matmul(kv[:], wkv_r[:, ck, :], rhs,
                                     start=(ck == 0), stop=(ck == NCK - 1))
                nc.vector.tensor_scalar(out=k_dup[0:64, sl], in0=kv[0:64, :],
                                        scalar1=kb_sb[:], scalar2=None, op0=ADD)
                nc.gpsimd.tensor_copy(out=k_dup[64:128, sl],
                                      in_=k_dup[0:64, sl].bitcast(F32))
            for kv, sl in kvs:
                nc.vector.tensor_copy(out=v_sb[64:128, sl], in_=kv[64:128, :])

        def vt_part(qt):
            for jt in range(qt * (NJT // NQT), (qt + 1) * (NJT // NQT)):
                pvt = bsh.tile([128, DH], F32, tag="b", name="pvt")
                nc.tensor.transpose(pvt[:], v_sb[64:128, jt * 128:(jt + 1) * 128],
                                    ident[64:128, 64:128])
                nc.vector.tensor_copy(out=vT_aug[:, jt, 0:DH], in_=pvt[:])

        # ---------------- attention blocks ----------------
        pav_tiles = {}

        def att_block(ic, qt):
            """QK + exp + AV for chunk ic over quarter qt's key tiles."""
            sl = slice(ic * IC, (ic + 1) * IC)
            if qt == 0:
                pav_tiles[ic] = psav.tile([DH + 1, IC], F32, tag="pav", name="pav")
            pav = pav_tiles[ic]
            for jp in range(qt * JPQ, (qt + 1) * JPQ):
                jA, jB = 2 * jp, 2 * jp + 1
                tt = pst.tile([128, 2, IC], F32, tag="tps", name="tps")
                nc.tensor.matmul(tt[:, 0, :], k_dup[0:64, jA * 128:(jA + 1) * 128],
                                 q_dup[0:64, sl], start=True, stop=True)
                nc.tensor.matmul(tt[:, 1, :], k_dup[64:128, jB * 128:(jB + 1) * 128],
                                 q_dup[64:128, sl], start=True, stop=True)
                ee = expp.tile([128, 2, IC], F32R, tag="exp", name="exp")
                nc.scalar.activation(out=ee[:], in_=tt[:],
                                     func=mybir.ActivationFunctionType.Exp)
                nc.tensor.matmul(pav[:], vT_aug[:, jA, :], ee[:, 0, :],
                                 start=(jp == 0), stop=False)
                nc.tensor.matmul(pav[:], vT_aug[:, jB, :], ee[:, 1, :],
                                 start=False, stop=(jp == NJT // 2 - 1))

        def att_finish(ic):
            """normalize + output projection + bias/residual + store."""
            sl = slice(ic * IC, (ic + 1) * IC)
            pav = pav_tiles.pop(ic)
            rden = attp.tile([1, IC], F32R, tag="rden", name="rden")
            with nc.allow_low_precision(reason="f32r matmul operand"):
                nc.vector.reciprocal(out=rden[:], in_=pav[64:65, :])
            pb = bsh.tile([DH, IC], F32, tag="b", name="pb")
            nc.tensor.matmul(pb[:], ones1[:], rden[:], start=True, stop=True)
            rb = attp.tile([DH, IC], F32, tag="rb", name="rb")
            nc.vector.tensor_copy(out=rb[:], in_=pb[:])
            attn = attp.tile([DH, IC], F32R, tag="attn", name="attn")
            nc.vector.tensor_tensor(out=attn[:], in0=pav[0:64, :], in1=rb[:], op=MUL)
            for ot in range(NCO):
                po = bsh.tile([128, IC], F32, tag="b", name="po")
                nc.tensor.matmul(po[:], woT_r[:, ot * 128:(ot + 1) * 128],
                                 attn[:], start=True, stop=True)
                xm = xmp.tile([128, IC], F32, tag="xm", name="xm")
                nc.gpsimd.tensor_scalar_mul(out=xm[:],
                                            in0=x_sb[:, ot, sl].bitcast(F32),
                                            scalar1=msk[:])
                pt = outp.tile([128, IC], F32, tag="pt", name="pt")
                nc.vector.scalar_tensor_tensor(out=pt[:], in0=po[:],
                                               scalar=bomv[:, ot:ot + 1],
                                               in1=xm[:], op0=ADD, op1=ADD)
                nc.sync.dma_start(out=pvw[:, ot, sl], in_=pt[:])

        # Emission: chunks 0 and 1 alternate per quarter right behind the
        # kv quarters (T-tile slots are FIFO per tag, so cross-chunk work
        # must be emitted in consumption order); remaining chunks run
        # sequentially with all key tiles available.  2 chunks in flight
        # (psav bufs=2).
        q_proj(0, pool=psav, tag="pav")
        nq = 2
        for qt in range(NQT):
            kv_quarter(qt)
            vt_part(qt)
            if qt == 0:
                q_proj(1, pool=psav, tag="pav")
            att_block(0, qt)
            att_block(1, qt)
            if nq < NIC:
                q_proj(nq)
                nq += 1
        att_finish(0)
        att_finish(1)
        for ic in range(2, NIC):
            if nq < NIC:
                q_proj(nq)
                nq += 1
            for qt in range(NQT):
                att_block(ic, qt)
            att_finish(ic)

        for p in [crp, xrp, stats_pool, ctxq, xmp, outp, attp, expp, bsh,
                  psav, pst, dram, singles]:
            p.release()

    nc.compile()
    return nc


GMAT = (np.arange(128)[:, None] // 16 == np.arange(8)[None, :]).astype(np.float32)
GMATT = np.ascontiguousarray(GMAT.T)
IDENT = np.eye(128, dtype=np.float32)

_NC_CACHE = None


def get_nc():
    global _NC_CACHE
    if _NC_CACHE is None:
        _NC_CACHE = build_nc()
    return _NC_CACHE


def kernel(x, context, gn_w, gn_b, wq, bq, wk, bk, wv, bv, wo, bo):
    from concourse.bass_utils import run_bass_kernel_spmd

    x = np.asarray(x, dtype=np.float32)
    context = np.asarray(context, dtype=np.float32)
    gn_w = np.asarray(gn_w, dtype=np.float32)
    gn_b = np.asarray(gn_b, dtype=np.float32)
    wq = np.asarray(wq, dtype=np.float32)
    bq = np.asarray(bq, dtype=np.float32)
    wk = np.asarray(wk, dtype=np.float32)
    bk = np.asarray(bk, dtype=np.float32)
    wv = np.asarray(wv, dtype=np.float32)
    bv = np.asarray(bv, dtype=np.float32)
    wo = np.asarray(wo, dtype=np.float32)
    bo = np.asarray(bo, dtype=np.float32)

    B, C, H, W = x.shape
    x2 = np.ascontiguousarray(x.reshape(C, H * W))
    ctx2 = np.ascontiguousarray(context.reshape(CTXC, H * W))

    in_maps = []
    for h in range(NH):
        hs = slice(h * DH, (h + 1) * DH)
        wkvT_h = np.concatenate([wk[hs, :].T, wv[hs, :].T], axis=1)
        in_maps.append({
            "x": x2,
            "ctx": ctx2,
            "gn_w": gn_w,
            "gn_b": gn_b,
            "wqT": np.ascontiguousarray(wq[hs, :].T),
            "wkvT": np.ascontiguousarray(wkvT_h),
            "woT": np.ascontiguousarray(wo[:, hs].T),
            "bq": np.ascontiguousarray(bq[hs]),
            "bk": np.ascontiguousarray(bk[hs]),
            "bv": np.ascontiguousarray(bv[hs]),
            "bo": bo,
            "is0": np.array([[1.0 if h == 0 else 0.0]], dtype=np.float32),
            "gmat": GMAT,
            "gmatT": GMATT,
            "ident": IDENT,
        })

    nc = get_nc()
    res = run_bass_kernel_spmd(nc, in_maps, core_ids=list(range(NH)))
    acc = np.zeros((C, H * W), dtype=np.float64)
    for h in range(NH):
        acc += res.results[h]["partial"]
    return acc.astype(np.float32).reshape(B, C, H, W)


# revision 50
# speedup vs baseline: 1.0047x; 1.0047x over previous
"""Trainium2 Bass kernel for a CrossAttentionBlock (GroupNorm + 8-head
cross-attention + output projection + residual).

Sharding: one attention head per NeuronCore (8 heads / 8 cores).  Each core
computes its head's partial output projection wo[:, h] @ attn_h; the host sums
the 8 partials (partial-sum unshard).  Residual and output bias are added on
core 0 only (mask input), so the host-side sum is a pure reduce.

Structure (single TileContext, dependency-scheduled):
 - GroupNorm is folded into a per-channel scale/offset applied to the Q
   projection weights (q = (wq*a) @ x + (wq@d + bq)).
 - All heavy matmuls run in float32r (full PE rate).  x/ctx are declared
   float32r in DRAM so the DMA load is the rounded producer (no cast passes).
 - Scores are computed transposed (T[j,i] = k_j . q_i, row-packed K=64
   pairs), exp on ScalarE, and the softmax denominator rides as a ones-column
   in the augmented V^T of the AV matmul.
 - K/V are produced in four column-quarters and the attention chunk work is
   emitted diagonally against the quarters so the ScalarE exp pipeline starts
   ~40us in and stays saturated.

Self-contained: hardcodes all shapes from the problem spec.
"""

import sys

sys.path.insert(0, "/opt/trn_rl_repo")

import numpy as np

import concourse.bass as bass
import concourse.tile as tile
from concourse import bacc, mybir

F32 = mybir.dt.float32
F32R = mybir.dt.float32r

CH = 512          # x channels
CTXC = 768        # context channels
N = 4096          # spatial positions (64*64)
NH = 8            # heads
DH = 64           # head dim
G = 32            # groupnorm groups
EPS = 1e-5
NCO = CH // 128   # x channel blocks (4)
NCK = CTXC // 128  # ctx channel blocks (6)
IC = 512          # query-chunk size
NIC = N // IC     # 8 query chunks
NJT = N // 128    # 32 key tiles
NQT = 4           # context column quarters
QW = N // NQT     # quarter width (1024)
JPQ = NJT // 2 // NQT  # key-tile pairs per quarter (4)
SCALE = 1.0 / 8.0  # 1/sqrt(DH)
# PROJ_F32: run Q/KV projections as plain fp32 matmuls (4 cyc/row on the PE,
# but exact) instead of f32r with raw DMA-loaded operands (full rate, but the
# PE rounds raw operands to 11 mantissa bits).
PROJ_F32 = False
# PROJ_CAST: cast x/ctx blocks to f32r with compute engines (Pool for x, DVE
# for ctx) before the f32r projection matmuls — engine-produced f32r operands
# take the full-precision PE path (raw DMA-loaded operands get rounded to 11
# mantissa bits).
PROJ_CAST = True

ADD = mybir.AluOpType.add
SUB = mybir.AluOpType.subtract
MUL = mybir.AluOpType.mult


def build_nc():
    nc = bacc.Bacc("TRN2", num_devices=8, debug=False)

    XDT = F32 if (PROJ_F32 or PROJ_CAST) else F32R
    x = nc.dram_tensor("x", (CH, N), XDT, kind="ExternalInput")
    ctx_t = nc.dram_tensor("ctx", (CTXC, N), XDT, kind="ExternalInput")
    gn_w = nc.dram_tensor("gn_w", (CH,), F32, kind="ExternalInput")
    gn_b = nc.dram_tensor("gn_b", (CH,), F32, kind="ExternalInput")
    wqT = nc.dram_tensor("wqT", (CH, DH), F32, kind="ExternalInput")
    wkvT = nc.dram_tensor("wkvT", (CTXC, 2 * DH), F32, kind="ExternalInput")
    woT = nc.dram_tensor("woT", (DH, CH), F32, kind="ExternalInput")
    bq = nc.dram_tensor("bq", (DH,), F32, kind="ExternalInput")
    bk = nc.dram_tensor("bk", (DH,), F32, kind="ExternalInput")
    bv = nc.dram_tensor("bv", (DH,), F32, kind="ExternalInput")
    bo = nc.dram_tensor("bo", (CH,), F32, kind="ExternalInput")
    is0 = nc.dram_tensor("is0", (1, 1), F32, kind="ExternalInput")
    gmat = nc.dram_tensor("gmat", (128, 8), F32, kind="ExternalInput")
    gmatT = nc.dram_tensor("gmatT", (8, 128), F32, kind="ExternalInput")
    ident_d = nc.dram_tensor("ident", (128, 128), F32, kind="ExternalInput")
    partial = nc.dram_tensor("partial", (CH, N), F32, kind="ExternalOutput")

    xv = x.rearrange("(co p) n -> p co n", p=128)
    pvw = partial.rearrange("(co p) n -> p co n", p=128)

    with tile.TileContext(nc) as tc:
        singles = tc.alloc_tile_pool(name="singles", bufs=1)
        dram = tc.alloc_tile_pool(name="dram", bufs=1, space="DRAM")
        # PSUM: pst 3 + pav 2 + pso 1 + bsh 2 = 8 banks
        pst = tc.alloc_tile_pool(name="pst", bufs=2, space="PSUM")
        psav = tc.alloc_tile_pool(name="psav", bufs=2, space="PSUM")
        bsh = tc.alloc_tile_pool(name="bsh", bufs=2, space="PSUM")
        expp = tc.alloc_tile_pool(name="expp", bufs=4)
        attp = tc.alloc_tile_pool(name="attp", bufs=2)
        outp = tc.alloc_tile_pool(name="outp", bufs=2)
        xmp = tc.alloc_tile_pool(name="xmp", bufs=2)
        ctxq = tc.alloc_tile_pool(name="ctxq", bufs=7)
        stats_pool = tc.alloc_tile_pool(name="stats", bufs=2)
        xrp = tc.alloc_tile_pool(name="xrp", bufs=2)
        crp = tc.alloc_tile_pool(name="crp", bufs=2)

        # ---------------- persistent SBUF tiles ----------------
        x_sb = singles.tile([128, NCO, N], XDT)
        q_dup = singles.tile([128, N], F32R)
        k_dup = singles.tile([128, N], F32R)
        v_sb = singles.tile([128, N], F32)       # only rows 64:128 used
        vT_aug = singles.tile([128, NJT, DH + 1], F32R)
        wqT_sb = singles.tile([128, NCO, DH], F32)
        wqs_sb = singles.tile([128, NCO, DH], F32 if PROJ_F32 else F32R)
        assert not (PROJ_F32 and PROJ_CAST)
        wkvT_sb = singles.tile([128, NCK, 2 * DH], F32)
        wkv_r = singles.tile([128, NCK, 2 * DH], F32 if PROJ_F32 else F32R)
        woT_sb = singles.tile([DH, CH], F32)
        woT_r = singles.tile([DH, CH], F32R)
        ident = singles.tile([128, 128], F32)
        ones1 = singles.tile([1, DH], F32R)
        gmat_sb = singles.tile([128, 8], F32)
        gmatT_sb = singles.tile([8, 128], F32)
        gnw_pc = singles.tile([128, NCO], F32)
        gnb_pc = singles.tile([128, NCO], F32)
        mvall = singles.tile([128, NCO, 2], F32)
        mv3 = singles.tile([128, NCO, 3], F32)
        gsm = singles.tile([8, NCO, 3], F32)
        gmu84 = singles.tile([8, NCO], F32)
        gvar84 = singles.tile([8, NCO], F32)
        srt84 = singles.tile([8, NCO], F32)
        grs = singles.tile([8, NCO, 2], F32)
        rg_pc = singles.tile([128, NCO, 2], F32)
        tmp_pc = singles.tile([128, NCO], F32)
        eps8 = singles.tile([8, 1], F32)
        a_pc = singles.tile([128, NCO], F32)
        d_pc = singles.tile([128, NCO], F32)
        qbias = singles.tile([DH, 1], F32)
        bq_sb = singles.tile([DH, 1], F32)
        kb_sb = singles.tile([DH, 1], F32)
        bv64 = singles.tile([DH, 1], F32)
        wobv_pc = singles.tile([128, NCO], F32)
        bomv = singles.tile([128, NCO], F32)
        bo_pc = singles.tile([128, NCO], F32)
        bom = singles.tile([128, NCO], F32)
        msk = singles.tile([128, 1], F32)

        # ---------------- input loads ----------------
        for co in range(NCO):
            nc.sync.dma_start(out=x_sb[:, co, :], in_=xv[:, co, :])
        nc.sync.dma_start(out=wqT_sb[:], in_=wqT.rearrange("(co p) d -> p co d", p=128))
        nc.sync.dma_start(out=wkvT_sb[:], in_=wkvT.rearrange("(ck p) d -> p ck d", p=128))
        nc.sync.dma_start(out=woT_sb[:], in_=woT[:])
        nc.sync.dma_start(out=gnw_pc[:], in_=gn_w.rearrange("(co p) -> p co", p=128))
        nc.sync.dma_start(out=gnb_pc[:], in_=gn_b.rearrange("(co p) -> p co", p=128))
        nc.sync.dma_start(out=gmat_sb[:], in_=gmat[:])
        nc.sync.dma_start(out=gmatT_sb[:], in_=gmatT[:])
        nc.sync.dma_start(out=ident[:], in_=ident_d[:])
        nc.sync.dma_start(out=bq_sb[:], in_=bq[:][:, None])
        nc.sync.dma_start(out=kb_sb[:], in_=bk[:][:, None])
        nc.sync.dma_start(out=bv64[:], in_=bv[:][:, None])
        nc.sync.dma_start(out=bo_pc[:], in_=bo.rearrange("(co p) -> p co", p=128))
        nc.sync.dma_start(out=msk[:], in_=is0[:].to_broadcast((128, 1)))

        onesf = singles.tile([128, 64], F32)
        nc.vector.memset(onesf[:], 1.0)
        expwarm = singles.tile([1, 1], F32)
        nc.vector.memset(expwarm[:], 1.0)
        nc.scalar.activation(out=expwarm[:], in_=expwarm[:],
                             func=mybir.ActivationFunctionType.Ln)
        nc.scalar.activation(out=expwarm[:], in_=expwarm[:],
                             func=mybir.ActivationFunctionType.Exp)
        nc.vector.tensor_copy(out=ones1[:], in_=onesf[0:1, 0:DH])
        nc.vector.tensor_copy(out=vT_aug[:, :, DH], in_=onesf[:, 0:NJT])
        nc.vector.memset(eps8[:], EPS)
        nc.vector.tensor_scalar_mul(out=bom[:], in0=bo_pc[:], scalar1=msk[:])
        nc.vector.tensor_copy(out=woT_r[:], in_=woT_sb[:])
        nc.vector.tensor_copy(out=wkv_r[:], in_=wkvT_sb[:])
        for ot in range(NCO):
            pwb = bsh.tile([128, 1], F32, tag="b", name="pwb")
            nc.tensor.matmul(pwb[:], woT_sb[:, ot * 128:(ot + 1) * 128], bv64[:],
                             start=True, stop=True)
            nc.vector.tensor_copy(out=wobv_pc[:, ot:ot + 1], in_=pwb[:])
        nc.vector.tensor_tensor(out=bomv[:], in0=bom[:], in1=wobv_pc[:], op=ADD)

        # ---------------- groupnorm statistics ----------------
        for co in range(NCO):
            st = stats_pool.tile([128, 8, 6], F32)
            xvw = x_sb[:, co, :].bitcast(F32).rearrange("p (s c) -> p s c", c=512)
            for s in range(8):
                nc.vector.bn_stats(out=st[:, s, :], in_=xvw[:, s, :])
            nc.vector.bn_aggr(out=mvall[:, co, :], in_=st[:])
        # group stats via tiny PE matmuls (no DRAM bounces):
        # gsm[j, co, :] = sum over the 16 channels of group (co*8+j) of
        # (mean, var, mean^2); then per-group mu/var and per-channel a/d.
        for co in range(NCO):
            nc.vector.tensor_copy(out=mv3[:, co, 0:2], in_=mvall[:, co, :])
            nc.vector.tensor_tensor(out=mv3[:, co, 2:3], in0=mvall[:, co, 0:1],
                                    in1=mvall[:, co, 0:1], op=MUL)
            psg = bsh.tile([8, 3], F32, tag="b", name="psg")
            nc.tensor.matmul(psg[:], gmat_sb[:], mv3[:, co, :],
                             start=True, stop=True)
            nc.vector.tensor_copy(out=gsm[:, co, :], in_=psg[:])
        nc.vector.tensor_scalar_mul(out=gmu84[:], in0=gsm[:, :, 0], scalar1=1.0 / 16.0)
        # gvar = E[var] + E[mean^2] - gmu^2
        nc.vector.tensor_tensor(out=gvar84[:], in0=gsm[:, :, 1], in1=gsm[:, :, 2],
                                op=ADD)
        nc.vector.tensor_scalar_mul(out=gvar84[:], in0=gvar84[:], scalar1=1.0 / 16.0)
        nc.vector.tensor_tensor(out=srt84[:], in0=gmu84[:], in1=gmu84[:], op=MUL)
        nc.vector.tensor_tensor(out=gvar84[:], in0=gvar84[:], in1=srt84[:], op=SUB)
        # rstd = exp(-0.5*ln(var+eps)): keeps the ScalarE activation table
        # on a set that also holds Exp.
        nc.scalar.activation(out=srt84[:], in_=gvar84[:],
                             func=mybir.ActivationFunctionType.Ln, bias=eps8[:])
        nc.scalar.activation(out=grs[:, :, 0], in_=srt84[:],
                             func=mybir.ActivationFunctionType.Exp, scale=-0.5)
        nc.vector.tensor_copy(out=grs[:, :, 1], in_=gmu84[:])
        for co in range(NCO):
            psr = bsh.tile([128, 2], F32, tag="b", name="psr")
            nc.tensor.matmul(psr[:], gmatT_sb[:], grs[:, co, :],
                             start=True, stop=True)
            nc.vector.tensor_copy(out=rg_pc[:, co, :], in_=psr[:])
        nc.vector.tensor_tensor(out=a_pc[:], in0=gnw_pc[:], in1=rg_pc[:, :, 0],
                                op=MUL)
        nc.vector.tensor_tensor(out=tmp_pc[:], in0=rg_pc[:, :, 1], in1=a_pc[:], op=MUL)
        nc.vector.tensor_tensor(out=d_pc[:], in0=gnb_pc[:], in1=tmp_pc[:], op=SUB)

        # qbias = wq_h @ d + bq ; wqs = wqT * a (columns scaled per channel)
        qb = psav.tile([DH, 1], F32, tag="pav", name="qb")
        for co in range(NCO):
            nc.tensor.matmul(qb[:], wqT_sb[:, co, :], d_pc[:, co:co + 1],
                             start=(co == 0), stop=(co == NCO - 1))
        nc.vector.tensor_tensor(out=qbias[:], in0=qb[:], in1=bq_sb[:], op=ADD)
        for co in range(NCO):
            nc.vector.tensor_scalar_mul(out=wqs_sb[:, co, :], in0=wqT_sb[:, co, :],
                                        scalar1=a_pc[:, co:co + 1])

        # ---------------- projections (f32r) ----------------
        def q_proj(ic, pool=None, tag="b"):
            sl = slice(ic * IC, (ic + 1) * IC)
            pq = (pool or bsh).tile([DH, IC], F32, tag=tag, name="pq")
            for co in range(NCO):
                if PROJ_CAST:
                    xr = xrp.tile([128, IC], F32R, tag="xr", name="xr")
                    nc.gpsimd.tensor_copy(out=xr[:], in_=x_sb[:, co, sl])
                    rhs = xr[:]
                else:
                    rhs = x_sb[:, co, sl]
                nc.tensor.matmul(pq[:], wqs_sb[:, co, :], rhs,
                                 start=(co == 0), stop=(co == NCO - 1))
            # q = (wq@xn + qbias) * SCALE, duplicated to partitions 64:128
            nc.vector.tensor_scalar(out=q_dup[0:64, sl], in0=pq[:],
                                    scalar1=qbias[:], scalar2=SCALE,
                                    op0=ADD, op1=MUL)
            nc.gpsimd.tensor_copy(out=q_dup[64:128, sl],
                                  in_=q_dup[0:64, sl].bitcast(F32))

        def kv_quarter(qt):
            cts = []
            for ck in range(NCK):
                ct = ctxq.tile([128, QW], XDT, tag="ctq", name="ctq")
                nc.sync.dma_start(
                    out=ct[:],
                    in_=ctx_t[ck * 128:(ck + 1) * 128, qt * QW:(qt + 1) * QW])
                cts.append(ct)
            kvs = []
            for lc in range(QW // IC):
                ic = qt * (QW // IC) + lc
                sl = slice(ic * IC, (ic + 1) * IC)
                kv = bsh.tile([128, IC], F32, tag="b", name="kv")
                kvs.append((kv, sl))
                for ck in range(NCK):
                    if PROJ_CAST:
                        cr = crp.tile([128, IC], F32R, tag="cr", name="cr")
                        nc.vector.tensor_copy(out=cr[:],
                                              in_=cts[ck][:, lc * IC:(lc + 1) * IC])
                        rhs = cr[:]
                    else:
                        rhs = cts[ck][:, lc * IC:(lc + 1) * IC]
                    nc.tensor.matmul(kv[:], wkv_r[:, ck, :], rhs,
                                     start=(ck == 0), stop=(ck == NCK - 1))
                nc.vector.tensor_scalar(out=k_dup[0:64, sl], in0=kv[0:64, :],
                                        scalar1=kb_sb[:], scalar2=None, op0=ADD)
                nc.gpsimd.tensor_copy(out=k_dup[64:128, sl],
                                      in_=k_dup[0:64, sl].bitcast(F32))
            for kv, sl in kvs:
                nc.vector.tensor_copy(out=v_sb[64:128, sl], in_=kv[64:128, :])

        def vt_part(qt):
            for jt in range(qt * (NJT // NQT), (qt + 1) * (NJT // NQT)):
                pvt = bsh.tile([128, DH], F32, tag="b", name="pvt")
                nc.tensor.transpose(pvt[:], v_sb[64:128, jt * 128:(jt + 1) * 128],
                                    ident[64:128, 64:128])
                nc.vector.tensor_copy(out=vT_aug[:, jt, 0:DH], in_=pvt[:])

        # ---------------- attention blocks ----------------
        pav_tiles = {}

        def att_block(ic, qt):
            """QK + exp + AV for chunk ic over quarter qt's key tiles."""
            sl = slice(ic * IC, (ic + 1) * IC)
            if qt == 0:
                pav_tiles[ic] = psav.tile([DH + 1, IC], F32, tag="pav", name="pav")
            pav = pav_tiles[ic]
            for jp in range(qt * JPQ, (qt + 1) * JPQ):
                jA, jB = 2 * jp, 2 * jp + 1
                tt = pst.tile([128, 2, IC], F32, tag="tps", name="tps")
                nc.tensor.matmul(tt[:, 0, :], k_dup[0:64, jA * 128:(jA + 1) * 128],
                                 q_dup[0:64, sl], start=True, stop=True)
                nc.tensor.matmul(tt[:, 1, :], k_dup[64:128, jB * 128:(jB + 1) * 128],
                                 q_dup[64:128, sl], start=True, stop=True)
                ee = expp.tile([128, 2, IC], F32R, tag="exp", name="exp")
                nc.scalar.activation(out=ee[:], in_=tt[:],
                                     func=mybir.ActivationFunctionType.Exp)
                nc.tensor.matmul(pav[:], vT_aug[:, jA, :], ee[:, 0, :],
                                 start=(jp == 0), stop=False)
                nc.tensor.matmul(pav[:], vT_aug[:, jB, :], ee[:, 1, :],
                                 start=False, stop=(jp == NJT // 2 - 1))

        def att_finish(ic):
            """normalize + output projection + bias/residual + store."""
            sl = slice(ic * IC, (ic + 1) * IC)
            pav = pav_tiles.pop(ic)
            rden = attp.tile([1, IC], F32R, tag="rden", name="rden")
            with nc.allow_low_precision(reason="f32r matmul operand"):
                nc.vector.reciprocal(out=rden[:], in_=pav[64:65, :])
            pb = bsh.tile([DH, IC], F32, tag="b", name="pb")
            nc.tensor.matmul(pb[:], ones1[:], rden[:], start=True, stop=True)
            rb = attp.tile([DH, IC], F32, tag="rb", name="rb")
            nc.vector.tensor_copy(out=rb[:], in_=pb[:])
            attn = attp.tile([DH, IC], F32R, tag="attn", name="attn")
            nc.vector.tensor_tensor(out=attn[:], in0=pav[0:64, :], in1=rb[:], op=MUL)
            for ot in range(NCO):
                po = bsh.tile([128, IC], F32, tag="b", name="po")
                nc.tensor.matmul(po[:], woT_r[:, ot * 128:(ot + 1) * 128],
                                 attn[:], start=True, stop=True)
                xm = xmp.tile([128, IC], F32, tag="xm", name="xm")
                nc.gpsimd.tensor_scalar_mul(out=xm[:],
                                            in0=x_sb[:, ot, sl].bitcast(F32),
                                            scalar1=msk[:])
                pt = outp.tile([128, IC], F32, tag="pt", name="pt")
                nc.vector.scalar_tensor_tensor(out=pt[:], in0=po[:],
                                               scalar=bomv[:, ot:ot + 1],
                                               in1=xm[:], op0=ADD, op1=ADD)
                nc.sync.dma_start(out=pvw[:, ot, sl], in_=pt[:])

        # Emission: chunks 0 and 1 alternate per quarter right behind the
        # kv quarters (T-tile slots are FIFO per tag, so cross-chunk work
        # must be emitted in consumption order); remaining chunks run
        # sequentially with all key tiles available.  2 chunks in flight
        # (psav bufs=2).
        q_proj(0, pool=psav, tag="pav")
        nq = 2
        for qt in range(NQT):
            kv_quarter(qt)
            vt_part(qt)
            if qt == 0:
                q_proj(1, pool=psav, tag="pav")
            att_block(0, qt)
            att_block(1, qt)
            if nq < NIC:
                q_proj(nq)
                nq += 1
        att_finish(0)
        att_finish(1)
        for ic in range(2, NIC):
            if nq < NIC:
                q_proj(nq)
                nq += 1
            for qt in range(NQT):
                att_block(ic, qt)
            att_finish(ic)

        for p in [crp, xrp, stats_pool, ctxq, xmp, outp, attp, expp, bsh,
                  psav, pst, dram, singles]:
            p.release()

    nc.compile()
    return nc


GMAT = (np.arange(128)[:, None] // 16 == np.arange(8)[None, :]).astype(np.float32)
GMATT = np.ascontiguousarray(GMAT.T)
IDENT = np.eye(128, dtype=np.float32)

_NC_CACHE = None


def get_nc():
    global _NC_CACHE
    if _NC_CACHE is None:
        _NC_CACHE = build_nc()
    return _NC_CACHE


def kernel(x, context, gn_w, gn_b, wq, bq, wk, bk, wv, bv, wo, bo):
    from concourse.bass_utils import run_bass_kernel_spmd

    x = np.asarray(x, dtype=np.float32)
    context = np.asarray(context, dtype=np.float32)
    gn_w = np.asarray(gn_w, dtype=np.float32)
    gn_b = np.asarray(gn_b, dtype=np.float32)
    wq = np.asarray(wq, dtype=np.float32)
    bq = np.asarray(bq, dtype=np.float32)
    wk = np.asarray(wk, dtype=np.float32)
    bk = np.asarray(bk, dtype=np.float32)
    wv = np.asarray(wv, dtype=np.float32)
    bv = np.asarray(bv, dtype=np.float32)
    wo = np.asarray(wo, dtype=np.float32)
    bo = np.asarray(bo, dtype=np.float32)

    B, C, H, W = x.shape
    x2 = np.ascontiguousarray(x.reshape(C, H * W))
    ctx2 = np.ascontiguousarray(context.reshape(CTXC, H * W))

    in_maps = []
    for h in range(NH):
        hs = slice(h * DH, (h + 1) * DH)
        wkvT_h = np.concatenate([wk[hs, :].T, wv[hs, :].T], axis=1)
        in_maps.append({
            "x": x2,
            "ctx": ctx2,
            "gn_w": gn_w,
            "gn_b": gn_b,
            "wqT": np.ascontiguousarray(wq[hs, :].T),
            "wkvT": np.ascontiguousarray(wkvT_h),
            "woT": np.ascontiguousarray(wo[:, hs].T),
            "bq": np.ascontiguousarray(bq[hs]),
            "bk": np.ascontiguousarray(bk[hs]),
            "bv": np.ascontiguousarray(bv[hs]),
            "bo": bo,
            "is0": np.array([[1.0 if h == 0 else 0.0]], dtype=np.float32),
            "gmat": GMAT,
            "gmatT": GMATT,
            "ident": IDENT,
        })

    nc = get_nc()
    res = run_bass_kernel_spmd(nc, in_maps, core_ids=list(range(NH)))
    acc = np.zeros((C, H * W), dtype=np.float64)
    for h in range(NH):
        acc += res.results[h]["partial"]
    return acc.astype(np.float32).reshape(B, C, H, W)
